# revision 28
# baseline (speedup 1.0000x reference)
"""Bass/Trainium2 kernel for a 2-block single-head causal transformer.

Strategy (8 NeuronCores): data-parallel over batch (B=4 -> 4 core pairs),
sequence-parallel within each pair. Each core owns the interleaved global
query tiles {2j + t} (t = core parity), so the instruction stream is
identical on every core; all per-core variation (embedded rows, causal
masks, vocab slice) is input data.

v2 design (vs the first working version):
- h0 arrives from the host already transposed -> zero PE transposes at start.
- Attention computes TRANSPOSED scores  scoresT[t, q] = k_t . q_q  so the
  post-softmax attn@v contraction (over t) needs NO transposes at all, and
  softmax runs max-free (scores are bounded ~20, exp is fp32-safe), removing
  the cross-range reduce_max. The softmax denominator l[q] is recovered with
  ones-vector matmuls on the PE and folded back in via a GPSIMD
  partition_broadcast + the (mandatory anyway) PSUM->SBUF drain multiply.
- K^T / V are exchanged pair-wise in column/row halves with AllGathers
  interleaved (k0, v0, k1, v1); attention streams over stored key tiles in
  AllGather-arrival order, so collective latency hides under score compute.
- Stored-slot indexing is rank-major (slot s = r*8 + m), which makes the
  instruction stream core-invariant; the one-block causal asymmetry between
  the two parities is absorbed by per-core mask data (head-block masks).
- Block-2's first K/V half (compute + AllGathers) is emitted in the middle
  of block-1's FFN so the collectives complete before block-2 attention.
- The scalar engine runs ONLY the softmax exp; all DMA issue lives on the
  sync/gpsimd queues (and vector/scalar once, for the initial weight load).
- The final-token logits machinery (w_out streaming, logits matmuls) is
  interleaved into the block-2 FFN so only a small drain remains at the end.

Everything is bf16 into the PE array with fp32 PSUM accumulation.
"""

import sys

sys.path.insert(0, "/opt/trn_rl_repo")

import numpy as np
import ml_dtypes

import concourse.bass as bass
import concourse.mybir as mybir
import concourse.tile as tile
from concourse import bacc
from concourse.bass_utils import run_bass_kernel_spmd
from concourse.masks import make_identity

BF16 = mybir.dt.bfloat16
F32 = mybir.dt.float32
P = 128
NEG = -30000.0


def build_nc(S=2048, D=1024, H=4096, V=32000, n_cores=8, stage="full"):
    """Build the SPMD Bass program (identical on all cores).

    stage: "h0" | "kv" | "att" | "block1" | "full" — truncate after the named
    phase and dump an intermediate to `dbg` (debug).
    """
    NJ = (S // P) // 2          # own q-tiles per core
    ND = D // P                 # d blocks
    NH = H // P                 # h blocks
    SO = S // 2                 # own rows per core
    NS = 2 * NJ                 # stored key tiles (both ranks), s = r*NJ + m
    VS = V // n_cores           # vocab slice per core
    KH = SO // 2                # kT columns per AllGather half
    MH = NJ // 2                # v row-tiles per AllGather half
    W1CH = 8                    # h-blocks per streamed w1 chunk
    VC = 500                    # logits n-chunk
    NVC = VS // VC              # logits chunks (8)
    pair_groups = [[2 * i, 2 * i + 1] for i in range(n_cores // 2)]
    all_group = [list(range(n_cores))]

    nc = bacc.Bacc("TRN2", target_bir_lowering=False, debug=False,
                   num_devices=n_cores)

    # ---- external inputs ----
    # h0T = (emb[tokens[own rows]] + pe[own rows]).T, staged on the host as
    # part of sharding (row gather + layout, no matmul compute)
    h0T = nc.dram_tensor("h0T", [ND, P, SO], BF16, kind="ExternalInput")
    # head-block masks, [r, t_row, q_col] (per-core data, see make_in_maps)
    maskT = nc.dram_tensor("maskT", [2, P, P], BF16, kind="ExternalInput")
    wts = {}
    for l in (1, 2):
        for nm in ("wk", "wv", "wo", "w1", "w2"):
            shp = [D, H] if nm == "w1" else ([H, D] if nm == "w2" else [D, D])
            wts[l, nm] = nc.dram_tensor(f"l{l}_{nm}", shp, BF16, kind="ExternalInput")
    w_out = nc.dram_tensor("w_out", [D, VS], BF16, kind="ExternalInput")
    logits = nc.dram_tensor("logits", [4, VS], F32, kind="ExternalOutput")
    dbg = None
    if stage != "full":
        dbg = nc.dram_tensor("dbg", [P, ND, S], BF16, kind="ExternalOutput")

    with tile.TileContext(nc) as tc:
        with (
            tc.tile_pool(name="sb", bufs=1) as sb,       # all SBUF, per-tag bufs
            tc.tile_pool(name="ps", bufs=1, space="PSUM") as ps_p,
            tc.tile_pool(name="dram", bufs=2, space="DRAM") as dram_p,
        ):
            # ---- constants ----
            mask_sb = sb.tile([P, 2, P], BF16, tag="mask")
            nc.sync.dma_start(mask_sb[:], maskT[:].rearrange("r t q -> t r q"))
            ones_sb = sb.tile([P, 1], BF16, tag="ones")
            nc.vector.memset(ones_sb[:], 1.0)
            ident = sb.tile([P, P], BF16, tag="ident")
            make_identity(nc, ident[:])

            warm_ctr = [0]

            def pe_warmup(n):
                # dummy N=128 matmuls (ident @ ident -> junk psum): keep the
                # PE HAM clock-gate warm through known-idle windows
                pj = ps_p.tile([P, 512], F32, tag="sc", bufs=2,
                               name=f"warm{warm_ctr[0]}")
                warm_ctr[0] += 1
                for _ in range(n):
                    nc.tensor.matmul(pj[:, :P], ident[:], ident[:],
                                     start=True, stop=True,
                                     skip_group_check=True)

            pe_warmup(60)

            # ---- initial loads: h0T + wk + wv interleaved over 4 queues so
            # the first K^T matmul group's operands land fast ----
            q4 = [nc.sync, nc.gpsimd, nc.scalar]
            own_hT = sb.tile([P, ND, SO], BF16, tag="own", bufs=2)
            wk_sb = sb.tile([P, ND, D], BF16, tag="wkv", bufs=2)
            wv_sb = sb.tile([P, ND, D], BF16, tag="wkv", bufs=2)
            for k in range(ND):
                q4[k % 3].dma_start(wk_sb[:, k, :], wts[1, "wk"][k * P : (k + 1) * P, :])
                q4[(k + 1) % 3].dma_start(own_hT[:, k, :], h0T[k])
            for k in range(ND):
                q4[(k + 2) % 3].dma_start(wv_sb[:, k, :], wts[1, "wv"][k * P : (k + 1) * P, :])
            wo_sb = sb.tile([P, ND, D], BF16, tag="wo")
            nc.sync.dma_start(wo_sb[:], wts[1, "wo"][:].rearrange("(k p) n -> p k n", p=P))

            if stage == "h0":
                nc.sync.dma_start(dbg[:, :, :SO], own_hT[:])

            st_ctr = [0]

            def stage_out(ps_ap, dst_ap):
                st = sb.tile([P, 512], BF16, tag="st", bufs=4, name=f"st{st_ctr[0]}")
                st_ctr[0] += 1
                nc.vector.tensor_copy(st[:], ps_ap)
                eng = nc.sync if st_ctr[0] % 2 == 0 else nc.gpsimd
                eng.dma_start(dst_ap, st[:])

            # per-block state
            S_ = {}

            def emit_cc_tiles(l):
                S_[l, "cc_in_k"] = [dram_p.tile([D, KH], BF16, tag=f"cck{h}", name=f"cck{h}_{l}") for h in range(2)]
                S_[l, "cc_out_k"] = [dram_p.tile([2, D, KH], BF16, tag=f"ccko{h}", name=f"ccko{h}_{l}") for h in range(2)]

            def emit_kT_full(l, hT, wk_ap):
                """own K^T, all columns, one pair AllGather (block-1 path:
                fewer collectives -> less per-op setup on the CC engine)."""
                cc_in = dram_p.tile([D, SO], BF16, tag="cck0", name=f"cckf_{l}")
                cc_out = dram_p.tile([2, D, SO], BF16, tag="ccko0", name=f"cckfo_{l}")
                S_[l, "cc_out_k_full"] = cc_out
                for hh in range(2):
                    for i in range(ND):
                        pk = ps_p.tile([P, 512], F32, tag="mm", bufs=2, name=f"pk{l}{hh}{i}")
                        for k in range(ND):
                            nc.tensor.matmul(
                                pk[:], wk_ap[:, k, i * P : (i + 1) * P],
                                hT[:, k, hh * KH : (hh + 1) * KH],
                                start=(k == 0), stop=(k == ND - 1),
                            )
                        stage_out(pk[:], cc_in[i * P : (i + 1) * P, hh * KH : (hh + 1) * KH])
                nc.gpsimd.collective_compute(
                    "AllGather", mybir.AluOpType.bypass,
                    replica_groups=pair_groups,
                    ins=[cc_in[:].opt()], outs=[cc_out[:].opt()],
                )

            def emit_kT_half(l, hh, hT, wk_ap):
                """own K^T columns [hh*KH, (hh+1)*KH] + pair AllGather."""
                for i in range(ND):
                    pk = ps_p.tile([P, 512], F32, tag="mm", bufs=2, name=f"pk{l}{hh}{i}")
                    for k in range(ND):
                        nc.tensor.matmul(
                            pk[:], wk_ap[:, k, i * P : (i + 1) * P],
                            hT[:, k, hh * KH : (hh + 1) * KH],
                            start=(k == 0), stop=(k == ND - 1),
                        )
                    stage_out(pk[:], S_[l, "cc_in_k"][hh][i * P : (i + 1) * P, :])
                nc.gpsimd.collective_compute(
                    "AllGather", mybir.AluOpType.bypass,
                    replica_groups=pair_groups,
                    ins=[S_[l, "cc_in_k"][hh][:].opt()],
                    outs=[S_[l, "cc_out_k"][hh][:].opt()],
                )

            def emit_v_tiles(l, hh, nm, hT, wv_ap):
                """own V row-tiles [hh*MH*nm, ...) + pair AllGather (nm halves)."""
                cc_in = dram_p.tile([nm * MH * P, D], BF16, tag=f"ccv{hh}", name=f"ccv{hh}_{l}")
                cc_out = dram_p.tile([2, nm * MH * P, D], BF16, tag=f"ccvo{hh}", name=f"ccvo{hh}_{l}")
                S_[l, "cc_out_v", hh] = cc_out
                for m0 in range(nm * MH):
                    m = hh * MH + m0
                    for ci in range(2):
                        pv = ps_p.tile([P, 512], F32, tag="mm", bufs=2, name=f"pv{l}{m}{ci}")
                        for k in range(ND):
                            nc.tensor.matmul(
                                pv[:], hT[:, k, m * P : (m + 1) * P],
                                wv_ap[:, k, ci * 512 : (ci + 1) * 512],
                                start=(k == 0), stop=(k == ND - 1),
                            )
                        stage_out(pv[:], cc_in[m0 * P : (m0 + 1) * P, ci * 512 : (ci + 1) * 512])
                nc.gpsimd.collective_compute(
                    "AllGather", mybir.AluOpType.bypass,
                    replica_groups=pair_groups,
                    ins=[cc_in[:].opt()], outs=[cc_out[:].opt()],
                )

            def emit_vall_fill(l, hh, mlo, mhi):
                # fill v_all slots [hh*MH+mlo, hh*MH+mhi) of both ranks from
                # the gather output (scalar queue: never upstream of staging)
                if (l, "v_all") not in S_:
                    S_[l, "v_all"] = sb.tile([P, NS, D], BF16, tag="vall", name=f"vall{l}")
                cc_out = S_[l, "cc_out_v", hh]
                for r in range(2):
                    eng = nc.sync if r == 0 else nc.scalar
                    eng.dma_start(
                        S_[l, "v_all"][:, r * NJ + hh * MH + mlo : r * NJ + hh * MH + mhi, :],
                        cc_out[r, mlo * P : mhi * P, :].rearrange("(m p) d -> p m d", p=P),
                    )

            def emit_attention(l, hT, kts_src, fills_mid=(), fills_end=()):
                """scoresT -> max-free exp -> l -> attn@v -> h_attnT."""
                slot_order = [r * NJ + m for m in range(MH) for r in range(2)] + \
                             [r * NJ + m for m in range(MH, NJ) for r in range(2)]
                expT = sb.tile([P, NS, SO], BF16, tag="expT", name=f"expT{l}")
                l_ps = [ps_p.tile([1, 512], F32, tag="lps", bufs=2, name=f"lps{l}{c}")
                        for c in range(2)]

                if l == 1:
                    pe_warmup(70)
                unit_ctr = [0]
                for si, s in enumerate(slot_order):
                    if si == 8:
                        for f in fills_mid:
                            emit_vall_fill(*f)
                    r, m = s // NJ, s % NJ
                    kTs = sb.tile([P, ND, P], BF16, tag="kts", bufs=2, name=f"kts{l}{s}")
                    eng = nc.sync if si % 2 == 0 else nc.gpsimd
                    eng.dma_start(kTs[:], kts_src(r, m))
                    units = [(m // MH, (m % MH) * P)]
                    if m // MH == 0:
                        units.append((1, 0))
                    for (c, off) in units:
                        utag = "sc" if unit_ctr[0] % 2 == 0 else "mm"
                        unit_ctr[0] += 1
                        sc = ps_p.tile([P, 512], F32, tag=utag, bufs=2, name=f"sc{l}{s}{c}")
                        for k in range(ND):
                            nc.tensor.matmul(
                                sc[:, off:], kTs[:, k, :],
                                hT[:, k, c * 512 + off : (c + 1) * 512],
                                start=(k == 0), stop=(k == ND - 1),
                            )
                        if c == m // MH:
                            # suffix head block: causal mask (data per core)
                            nc.vector.tensor_add(
                                sc[:, off : off + P], sc[:, off : off + P],
                                mask_sb[:, r, :],
                            )
                        nc.scalar.activation(
                            expT[:, s, c * 512 + off : (c + 1) * 512], sc[:, off:],
                            mybir.ActivationFunctionType.Exp,
                        )
                        nc.tensor.matmul(
                            l_ps[c][:, off:], ones_sb[:],
                            expT[:, s, c * 512 + off : (c + 1) * 512],
                            start=(si == 0), stop=(si == len(slot_order) - 1),
                            skip_group_check=True,
                        )

                for f in fills_end:
                    emit_vall_fill(*f)

                inv_bc = []
                for c in range(2):
                    inv_sb = sb.tile([1, 512], F32, tag="inv", bufs=1, name=f"inv{l}{c}")
                    nc.vector.reciprocal(inv_sb[:], l_ps[c][:])
                    bc = sb.tile([P, 512], F32, tag="invbc", bufs=2, name=f"invbc{l}{c}")
                    nc.gpsimd.partition_broadcast(bc[:], inv_sb[:])
                    inv_bc.append(bc)

                v_all = S_[l, "v_all"]
                h_attnT = sb.tile([P, ND, SO], BF16, tag="hat", name=f"hat{l}")
                for c in range(2):
                    slots_c = [s for s in slot_order if (s % NJ) // MH <= c]
                    for i in range(ND):
                        av = ps_p.tile([P, 512], F32, tag="av", bufs=2, name=f"av{l}{c}{i}")
                        for sj, s in enumerate(slots_c):
                            m = s % NJ
                            off = (m % MH) * P if (m // MH) == c else 0
                            nc.tensor.matmul(
                                av[:, off:],
                                v_all[:, s, i * P : (i + 1) * P],
                                expT[:, s, c * 512 + off : (c + 1) * 512],
                                start=(sj == 0), stop=(sj == len(slots_c) - 1),
                                skip_group_check=True,
                            )
                        nc.vector.tensor_mul(
                            h_attnT[:, i, c * 512 : (c + 1) * 512], av[:], inv_bc[c][:],
                        )
                return h_attnT

            def emit_wo(l, hT, h_attnT, wo_ap):
                h_resT = sb.tile([P, ND, SO], BF16, tag="hres", name=f"hres{l}")
                for i in range(ND):
                    for c in range(2):
                        po = ps_p.tile([P, 512], F32, tag="mm", bufs=2, name=f"po{l}{i}{c}")
                        for k in range(ND):
                            nc.tensor.matmul(
                                po[:], wo_ap[:, k, i * P : (i + 1) * P],
                                h_attnT[:, k, c * 512 : (c + 1) * 512],
                                start=(k == 0), stop=(k == ND - 1),
                            )
                        nc.vector.tensor_add(
                            h_resT[:, i, c * 512 : (c + 1) * 512], po[:],
                            hT[:, i, c * 512 : (c + 1) * 512],
                        )
                return h_resT

            def emit_ffn_chunk(l, c, h_resT, own_next, last_col=None,
                               pre_w2=None, post_w1=None, post_w2_i=None):
                qoff = c * 512
                midT = sb.tile([P, NH, 512], BF16, tag="vall", name=f"midT{l}{c}")
                for ch in range(NH // W1CH):
                    w1_sb = sb.tile([P, ND, W1CH * P], BF16, tag="wkv", bufs=2,
                                    name=f"w1_{l}{c}{ch}")
                    nc.sync.dma_start(
                        w1_sb[:],
                        wts[l, "w1"][:, ch * W1CH * P : (ch + 1) * W1CH * P]
                        .rearrange("(k p) n -> p k n", p=P),
                    )
                    for hb in range(W1CH):
                        g = ch * W1CH + hb
                        pm = ps_p.tile([P, 512], F32, tag="mm", bufs=2,
                                       name=f"pm{l}{c}{g}")
                        for k in range(ND):
                            nc.tensor.matmul(
                                pm[:], w1_sb[:, k, hb * P : (hb + 1) * P],
                                h_resT[:, k, qoff : qoff + 512],
                                start=(k == 0), stop=(k == ND - 1),
                            )
                        nc.vector.tensor_scalar_max(midT[:, g, :], pm[:], 0.0)
                if post_w1 is not None:
                    post_w1()
                if pre_w2 is not None:
                    pre_w2()
                for i in range(ND):
                    w2_sb = sb.tile([P, NH, P], BF16, tag="kts", bufs=2,
                                    name=f"w2_{l}{c}{i}")
                    nc.gpsimd.dma_start(
                        w2_sb[:],
                        wts[l, "w2"][:, i * P : (i + 1) * P]
                        .rearrange("(k p) n -> p k n", p=P),
                    )
                    pw = ps_p.tile([P, 512], F32, tag="mm", bufs=2, name=f"pw{l}{c}{i}")
                    for hb in range(NH):
                        nc.tensor.matmul(
                            pw[:], w2_sb[:, hb, :], midT[:, hb, :],
                            start=(hb == 0), stop=(hb == NH - 1),
                        )
                    nc.vector.tensor_add(
                        own_next[:, i, qoff : qoff + 512], pw[:],
                        h_resT[:, i, qoff : qoff + 512],
                    )
                    if last_col is not None:
                        nc.vector.tensor_add(
                            last_col[:, i : i + 1], pw[:, 511:512],
                            h_resT[:, i, SO - 1 : SO],
                        )
                    if post_w2_i is not None:
                        post_w2_i(i)

            # ================= block 1 =================
            emit_cc_tiles(1)
            emit_kT_half(1, 0, own_hT, wk_sb)
            emit_v_tiles(1, 0, 1, own_hT, wv_sb)
            emit_kT_half(1, 1, own_hT, wk_sb)
            emit_v_tiles(1, 1, 1, own_hT, wv_sb)
            if stage == "kv":
                for s in range(NS):
                    r, m = s // NJ, s % NJ
                    nc.sync.dma_start(
                        dbg[:, :, s * P : (s + 1) * P],
                        S_[1, "cc_out_k"][m // MH][r, :, (m % MH) * P : (m % MH + 1) * P]
                        .rearrange("(i p) t -> p i t", p=P),
                    )
            elif stage in ("att", "block1", "full"):
                k1half = S_[1, "cc_out_k"]
                h_attnT = emit_attention(
                    1, own_hT,
                    lambda r, m: k1half[m // MH][r, :, (m % MH) * P : (m % MH + 1) * P]
                    .rearrange("(i p) t -> p i t", p=P),
                    fills_mid=[(1, 0, 0, 4)],
                    fills_end=[(1, 1, 0, 4)])
                if stage == "att":
                    nc.sync.dma_start(dbg[:, :, :SO], h_attnT[:])
                else:
                    h_resT = emit_wo(1, own_hT, h_attnT, wo_sb)
                    # prefetch block-2 wo while the FFN runs
                    wo2_sb = sb.tile([P, ND, D], BF16, tag="wo", name="wo2")
                    nc.sync.dma_start(wo2_sb[:], wts[2, "wo"][:].rearrange("(k p) n -> p k n", p=P))
                    own2 = sb.tile([P, ND, SO], BF16, tag="own", bufs=2, name="own2")

                    wkv2 = sb.tile([P, 2, ND, D], BF16, tag="expT", name="wkv2")

                    def prefetch_wkv2():
                        # scalar queue: idle during the FFN (exps all done),
                        # so this 4MB load contends with nothing
                        for k in range(ND):
                            nc.scalar.dma_start(wkv2[:, 0, k, :], wts[2, "wk"][k * P : (k + 1) * P, :])
                            nc.scalar.dma_start(wkv2[:, 1, k, :], wts[2, "wv"][k * P : (k + 1) * P, :])

                    emit_ffn_chunk(1, 0, h_resT, own2, pre_w2=prefetch_wkv2)
                    if stage == "full":
                        # block-2 K/V first half mid-FFN: its AllGathers run
                        # under block-1's remaining FFN work
                        emit_cc_tiles(2)
                        emit_kT_half(2, 0, own2, wkv2[:, 0])
                        emit_v_tiles(2, 0, 1, own2, wkv2[:, 1])
                    emit_ffn_chunk(1, 1, h_resT, own2)
                    if stage == "block1":
                        nc.sync.dma_start(dbg[:, :, :SO], own2[:])

            # ================= block 2 =================
            if stage == "full":
                emit_vall_fill(2, 0, 0, 4)
                emit_kT_half(2, 1, own2, wkv2[:, 0])
                emit_v_tiles(2, 1, 1, own2, wkv2[:, 1])
                k2half = S_[2, "cc_out_k"]
                h_attnT2 = emit_attention(
                    2, own2,
                    lambda r, m: k2half[m // MH][r, :, (m % MH) * P : (m % MH + 1) * P]
                    .rearrange("(i p) t -> p i t", p=P),
                    fills_end=[(2, 1, 0, 4)])
                h_resT2 = emit_wo(2, own2, h_attnT2, wo2_sb)
                own3 = sb.tile([P, ND, SO], BF16, tag="own", bufs=2, name="own3")
                last_col = sb.tile([P, ND], BF16, tag="lastcol")

                # FFN half with the final token first; AllGather of the last
                # token's activations overlaps the other half
                emit_ffn_chunk(2, 1, h_resT2, own3, last_col=last_col)
                lc_t = sb.tile([ND, P], BF16, tag="lct")
                ps_lc = ps_p.tile([P, P], BF16, tag="sc", bufs=2, name="pslc")
                nc.tensor.transpose(ps_lc[:ND, :P], last_col[:], ident[:])
                nc.vector.tensor_copy(lc_t[:], ps_lc[:ND, :P])
                cc_l_in = dram_p.tile([D], BF16, tag="ccl")
                cc_l_out = dram_p.tile([n_cores, D], BF16, tag="cclo")
                nc.sync.dma_start(cc_l_in[:].rearrange("(i p) -> i p", p=P), lc_t[:])
                nc.gpsimd.collective_compute(
                    "AllGather", mybir.AluOpType.bypass,
                    replica_groups=all_group,
                    ins=[cc_l_in[:].opt()], outs=[cc_l_out[:].opt()],
                )
                # gather read issued ahead of the 4MB w_out stream on the
                # scalar queue so the lhsT transposes never stall
                h_last = sb.tile([4, ND, P], BF16, tag="hlast")
                nc.scalar.dma_start(
                    h_last[:],
                    cc_l_out[:].rearrange("r (i p) -> r i p", p=P)[1::2],
                )
                S_["h_last"] = h_last

                lhsT = sb.tile([P, ND, 4], BF16, tag="lhsT")
                lg_ctr = [0]

                def logits_chunks(lo, hi, wo_stream):
                    for vc in range(lo, hi):
                        pl = ps_p.tile([P, 512], F32, tag="av", bufs=2, name=f"pl{vc}")
                        for k in range(ND):
                            nc.tensor.matmul(
                                pl[:4, :VC], lhsT[:, k, :],
                                wo_stream[:, k, (vc - lo) * VC : (vc - lo + 1) * VC],
                                start=(k == 0), stop=(k == ND - 1),
                            )
                        lg = sb.tile([4, VC], F32, tag="lg", bufs=1, name=f"lg{vc}")
                        nc.vector.tensor_copy(lg[:], pl[:4, :VC])
                        nc.scalar.dma_start(logits[:, vc * VC : (vc + 1) * VC], lg[:])

                wo_ta = sb.tile([P, ND, 4 * VC], BF16, tag="expT", name="wo_ta")
                nc.scalar.dma_start(
                    wo_ta[:],
                    w_out[:, : 4 * VC].rearrange("(k p) n -> p k n", p=P),
                )
                wo_tb = None

                def logits_mid():
                    # lhsT prep + first half of the vocab chunks, interleaved
                    # between w1 and w2 of the last FFN chunk (the 8-core
                    # AllGather is long done by now)
                    nonlocal wo_tb
                    h_last = S_["h_last"]
                    for i in range(ND):
                        ps_t = ps_p.tile([P, 512], BF16, tag="sc", bufs=2, name=f"pst{i}")
                        nc.tensor.transpose(ps_t[:, :4], h_last[:, i, :], ident[:4, :4])
                        nc.vector.tensor_copy(lhsT[:, i, :], ps_t[:, :4])
                    logits_chunks(0, 4, wo_ta)
                    wo_tb = sb.tile([P, ND, 4 * VC], BF16, tag="expT", name="wo_tb")
                    nc.scalar.dma_start(
                        wo_tb[:],
                        w_out[:, 4 * VC :].rearrange("(k p) n -> p k n", p=P),
                    )

                def logits_late(i):
                    if i == 5:
                        logits_chunks(4, NVC, wo_tb)

                emit_ffn_chunk(2, 0, h_resT2, own3, post_w1=logits_mid,
                               post_w2_i=logits_late)

    nc.compile()
    return nc


# ----------------------------------------------------------------------------
# host side
# ----------------------------------------------------------------------------

def make_in_maps(tokens, emb, pe, weights, S=2048, D=1024, H=4096, V=32000,
                 n_cores=8):
    """weights: dict with l{1,2}_{wk,wv,wo,w1,w2} and w_out (fp32 numpy)."""
    bf = ml_dtypes.bfloat16
    NJ = (S // P) // 2
    ND = D // P
    SO = S // 2
    VS = V // n_cores
    emb_f = np.ascontiguousarray(emb, dtype=np.float32)
    pe_f = np.asarray(pe, dtype=np.float32)
    scale = 1.0 / np.sqrt(float(D))
    w_bf = {}
    for l in (1, 2):
        w_bf[f"l{l}_wk"] = (np.asarray(weights[f"l{l}_wk"], np.float32) * scale).astype(bf)
        for nm in ("wv", "wo", "w1", "w2"):
            w_bf[f"l{l}_{nm}"] = np.asarray(weights[f"l{l}_{nm}"], np.float32).astype(bf)
    w_out_bf = np.asarray(weights["w_out"], np.float32).astype(bf)

    tokens = np.asarray(tokens)
    in_maps = []
    # scoresT head-block masks: [t_row, q_col] within the diagonal 128-tile;
    # allowed iff q >= t  ->  NEG on the strict lower triangle
    tri = np.tril(np.full((P, P), NEG, np.float32), k=-1)
    for c in range(n_cores):
        b, t = c // 2, c % 2
        own_rows = np.concatenate(
            [np.arange((2 * j + t) * P, (2 * j + t + 1) * P) for j in range(NJ)]
        )
        h0 = (emb_f[tokens[b, own_rows]] + pe_f[own_rows]).astype(np.float32)
        h0T = np.ascontiguousarray(h0.T).reshape(ND, P, SO).astype(bf)
        # head-block mask per stored rank r: global q-tile 2m+t vs key tile
        # 2m+r: t==r -> diagonal triangle; t<r -> fully masked; t>r -> allowed
        maskT = np.zeros((2, P, P), np.float32)
        for r in range(2):
            if t == r:
                maskT[r] = tri
            elif t < r:
                maskT[r] = NEG
        in_map = {
            "h0T": h0T,
            "maskT": maskT.astype(bf),
            "w_out": np.ascontiguousarray(w_out_bf[:, c * VS : (c + 1) * VS]),
        }
        in_map.update(w_bf)
        in_maps.append(in_map)
    return in_maps


_NC_CACHE = {}


def _get_nc(key=(2048, 1024, 4096, 32000, 8)):
    if key not in _NC_CACHE:
        _NC_CACHE[key] = build_nc(*key)
    return _NC_CACHE[key]


def kernel(tokens, emb, pe, l1_wk, l1_wv, l1_wo, l1_w1, l1_w2,
           l2_wk, l2_wv, l2_wo, l2_w1, l2_w2, w_out):
    S = int(np.asarray(tokens).shape[1])
    D = int(np.asarray(emb).shape[1])
    H = int(np.asarray(l1_w1).shape[1])
    V = int(np.asarray(emb).shape[0])
    n_cores = 8
    nc = _get_nc((S, D, H, V, n_cores))
    weights = dict(
        l1_wk=l1_wk, l1_wv=l1_wv, l1_wo=l1_wo, l1_w1=l1_w1, l1_w2=l1_w2,
        l2_wk=l2_wk, l2_wv=l2_wv, l2_wo=l2_wo, l2_w1=l2_w1, l2_w2=l2_w2,
        w_out=w_out,
    )
    in_maps = make_in_maps(tokens, emb, pe, weights, S, D, H, V, n_cores)
    try:
        res = run_bass_kernel_spmd(nc, in_maps, core_ids=list(range(n_cores)))
    except Exception:
        # a previous crashed run can leave the device wedged; one retry
        # (fresh NRT session) clears it
        import os
        os.environ.setdefault("NEURON_RT_RESET_CORES", "1")
        res = run_bass_kernel_spmd(nc, in_maps, core_ids=list(range(n_cores)))
    VS = V // n_cores
    out = np.zeros((np.asarray(tokens).shape[0], V), np.float32)
    for c in range(n_cores):
        out[:, c * VS : (c + 1) * VS] = res.results[c]["logits"]
    return out


# revision 29
# speedup vs baseline: 1.0018x; 1.0018x over previous
"""Bass/Trainium2 kernel for a 2-block single-head causal transformer.

Strategy (8 NeuronCores): data-parallel over batch (B=4 -> 4 core pairs),
sequence-parallel within each pair. Each core owns the interleaved global
query tiles {2j + t} (t = core parity), so the instruction stream is
identical on every core; all per-core variation (embedded rows, causal
masks, vocab slice) is input data.

v2 design (vs the first working version):
- h0 arrives from the host already transposed -> zero PE transposes at start.
- Attention computes TRANSPOSED scores  scoresT[t, q] = k_t . q_q  so the
  post-softmax attn@v contraction (over t) needs NO transposes at all, and
  softmax runs max-free (scores are bounded ~20, exp is fp32-safe), removing
  the cross-range reduce_max. The softmax denominator l[q] is recovered with
  ones-vector matmuls on the PE and folded back in via a GPSIMD
  partition_broadcast + the (mandatory anyway) PSUM->SBUF drain multiply.
- K^T / V are exchanged pair-wise in column/row halves with AllGathers
  interleaved (k0, v0, k1, v1); attention streams over stored key tiles in
  AllGather-arrival order, so collective latency hides under score compute.
- Stored-slot indexing is rank-major (slot s = r*8 + m), which makes the
  instruction stream core-invariant; the one-block causal asymmetry between
  the two parities is absorbed by per-core mask data (head-block masks).
- Block-2's first K/V half (compute + AllGathers) is emitted in the middle
  of block-1's FFN so the collectives complete before block-2 attention.
- The scalar engine runs ONLY the softmax exp; all DMA issue lives on the
  sync/gpsimd queues (and vector/scalar once, for the initial weight load).
- The final-token logits machinery (w_out streaming, logits matmuls) is
  interleaved into the block-2 FFN so only a small drain remains at the end.

Everything is bf16 into the PE array with fp32 PSUM accumulation.
"""

import sys

sys.path.insert(0, "/opt/trn_rl_repo")

import numpy as np
import ml_dtypes

import concourse.bass as bass
import concourse.mybir as mybir
import concourse.tile as tile
from concourse import bacc
from concourse.bass_utils import run_bass_kernel_spmd
from concourse.masks import make_identity

BF16 = mybir.dt.bfloat16
F32 = mybir.dt.float32
P = 128
NEG = -30000.0


def build_nc(S=2048, D=1024, H=4096, V=32000, n_cores=8, stage="full"):
    """Build the SPMD Bass program (identical on all cores).

    stage: "h0" | "kv" | "att" | "block1" | "full" — truncate after the named
    phase and dump an intermediate to `dbg` (debug).
    """
    NJ = (S // P) // 2          # own q-tiles per core
    ND = D // P                 # d blocks
    NH = H // P                 # h blocks
    SO = S // 2                 # own rows per core
    NS = 2 * NJ                 # stored key tiles (both ranks), s = r*NJ + m
    VS = V // n_cores           # vocab slice per core
    KH = SO // 2                # kT columns per AllGather half
    MH = NJ // 2                # v row-tiles per AllGather half
    W1CH = 8                    # h-blocks per streamed w1 chunk
    VC = 500                    # logits n-chunk
    NVC = VS // VC              # logits chunks (8)
    pair_groups = [[2 * i, 2 * i + 1] for i in range(n_cores // 2)]
    all_group = [list(range(n_cores))]

    nc = bacc.Bacc("TRN2", target_bir_lowering=False, debug=False,
                   num_devices=n_cores)

    # ---- external inputs ----
    # h0T = (emb[tokens[own rows]] + pe[own rows]).T, staged on the host as
    # part of sharding (row gather + layout, no matmul compute)
    h0T = nc.dram_tensor("h0T", [ND, P, SO], BF16, kind="ExternalInput")
    # head-block masks, [r, t_row, q_col] (per-core data, see make_in_maps)
    maskT = nc.dram_tensor("maskT", [2, P, P], BF16, kind="ExternalInput")
    wts = {}
    for l in (1, 2):
        for nm in ("wk", "wv", "wo", "w1", "w2"):
            shp = [D, H] if nm == "w1" else ([H, D] if nm == "w2" else [D, D])
            wts[l, nm] = nc.dram_tensor(f"l{l}_{nm}", shp, BF16, kind="ExternalInput")
    w_out = nc.dram_tensor("w_out", [D, VS], BF16, kind="ExternalInput")
    logits = nc.dram_tensor("logits", [4, VS], F32, kind="ExternalOutput")
    dbg = None
    if stage != "full":
        dbg = nc.dram_tensor("dbg", [P, ND, S], BF16, kind="ExternalOutput")

    with tile.TileContext(nc) as tc:
        with (
            tc.tile_pool(name="sb", bufs=1) as sb,       # all SBUF, per-tag bufs
            tc.tile_pool(name="ps", bufs=1, space="PSUM") as ps_p,
            tc.tile_pool(name="dram", bufs=2, space="DRAM") as dram_p,
        ):
            # ---- constants ----
            mask_sb = sb.tile([P, 2, P], BF16, tag="mask")
            nc.sync.dma_start(mask_sb[:], maskT[:].rearrange("r t q -> t r q"))
            ones_sb = sb.tile([P, 1], BF16, tag="ones")
            nc.vector.memset(ones_sb[:], 1.0)
            ident = sb.tile([P, P], BF16, tag="ident")
            make_identity(nc, ident[:])

            warm_ctr = [0]

            def pe_warmup(n):
                # dummy N=128 matmuls (ident @ ident -> junk psum): keep the
                # PE HAM clock-gate warm through known-idle windows
                pj = ps_p.tile([P, 512], F32, tag="sc", bufs=2,
                               name=f"warm{warm_ctr[0]}")
                warm_ctr[0] += 1
                for _ in range(n):
                    nc.tensor.matmul(pj[:, :P], ident[:], ident[:],
                                     start=True, stop=True,
                                     skip_group_check=True)

            pe_warmup(60)

            # ---- initial loads: h0T + wk + wv interleaved over 4 queues so
            # the first K^T matmul group's operands land fast ----
            q4 = [nc.sync, nc.gpsimd, nc.scalar]
            own_hT = sb.tile([P, ND, SO], BF16, tag="own", bufs=2)
            wk_sb = sb.tile([P, ND, D], BF16, tag="wkv", bufs=2)
            wv_sb = sb.tile([P, ND, D], BF16, tag="wkv", bufs=2)
            for k in range(ND):
                q4[k % 3].dma_start(wk_sb[:, k, :], wts[1, "wk"][k * P : (k + 1) * P, :])
                q4[(k + 1) % 3].dma_start(own_hT[:, k, :], h0T[k])
            for k in range(ND):
                q4[(k + 2) % 3].dma_start(wv_sb[:, k, :], wts[1, "wv"][k * P : (k + 1) * P, :])
            wo_sb = sb.tile([P, ND, D], BF16, tag="wo")
            nc.sync.dma_start(wo_sb[:], wts[1, "wo"][:].rearrange("(k p) n -> p k n", p=P))

            if stage == "h0":
                nc.sync.dma_start(dbg[:, :, :SO], own_hT[:])

            st_ctr = [0]

            def stage_out(ps_ap, dst_ap):
                st = sb.tile([P, 512], BF16, tag="st", bufs=4, name=f"st{st_ctr[0]}")
                st_ctr[0] += 1
                nc.vector.tensor_copy(st[:], ps_ap)
                eng = nc.sync if st_ctr[0] % 2 == 0 else nc.gpsimd
                eng.dma_start(dst_ap, st[:])

            # per-block state
            S_ = {}

            def emit_cc_tiles(l):
                S_[l, "cc_in_k"] = [dram_p.tile([D, KH], BF16, tag=f"cck{h}", name=f"cck{h}_{l}") for h in range(2)]
                S_[l, "cc_out_k"] = [dram_p.tile([2, D, KH], BF16, tag=f"ccko{h}", name=f"ccko{h}_{l}") for h in range(2)]

            def emit_kT_full(l, hT, wk_ap):
                """own K^T, all columns, one pair AllGather (block-1 path:
                fewer collectives -> less per-op setup on the CC engine)."""
                cc_in = dram_p.tile([D, SO], BF16, tag="cck0", name=f"cckf_{l}")
                cc_out = dram_p.tile([2, D, SO], BF16, tag="ccko0", name=f"cckfo_{l}")
                S_[l, "cc_out_k_full"] = cc_out
                for hh in range(2):
                    for i in range(ND):
                        pk = ps_p.tile([P, 512], F32, tag="mm", bufs=2, name=f"pk{l}{hh}{i}")
                        for k in range(ND):
                            nc.tensor.matmul(
                                pk[:], wk_ap[:, k, i * P : (i + 1) * P],
                                hT[:, k, hh * KH : (hh + 1) * KH],
                                start=(k == 0), stop=(k == ND - 1),
                            )
                        stage_out(pk[:], cc_in[i * P : (i + 1) * P, hh * KH : (hh + 1) * KH])
                nc.gpsimd.collective_compute(
                    "AllGather", mybir.AluOpType.bypass,
                    replica_groups=pair_groups,
                    ins=[cc_in[:].opt()], outs=[cc_out[:].opt()],
                )

            def emit_kT_half(l, hh, hT, wk_ap):
                """own K^T columns [hh*KH, (hh+1)*KH] + pair AllGather."""
                for i in range(ND):
                    pk = ps_p.tile([P, 512], F32, tag="mm", bufs=2, name=f"pk{l}{hh}{i}")
                    for k in range(ND):
                        nc.tensor.matmul(
                            pk[:], wk_ap[:, k, i * P : (i + 1) * P],
                            hT[:, k, hh * KH : (hh + 1) * KH],
                            start=(k == 0), stop=(k == ND - 1),
                        )
                    stage_out(pk[:], S_[l, "cc_in_k"][hh][i * P : (i + 1) * P, :])
                nc.gpsimd.collective_compute(
                    "AllGather", mybir.AluOpType.bypass,
                    replica_groups=pair_groups,
                    ins=[S_[l, "cc_in_k"][hh][:].opt()],
                    outs=[S_[l, "cc_out_k"][hh][:].opt()],
                )

            def emit_v_tiles(l, hh, nm, hT, wv_ap):
                """own V row-tiles [hh*MH*nm, ...) + pair AllGather (nm halves)."""
                cc_in = dram_p.tile([nm * MH * P, D], BF16, tag=f"ccv{hh}", name=f"ccv{hh}_{l}")
                cc_out = dram_p.tile([2, nm * MH * P, D], BF16, tag=f"ccvo{hh}", name=f"ccvo{hh}_{l}")
                S_[l, "cc_out_v", hh] = cc_out
                for m0 in range(nm * MH):
                    m = hh * MH + m0
                    for ci in range(2):
                        pv = ps_p.tile([P, 512], F32, tag="mm", bufs=2, name=f"pv{l}{m}{ci}")
                        for k in range(ND):
                            nc.tensor.matmul(
                                pv[:], hT[:, k, m * P : (m + 1) * P],
                                wv_ap[:, k, ci * 512 : (ci + 1) * 512],
                                start=(k == 0), stop=(k == ND - 1),
                            )
                        stage_out(pv[:], cc_in[m0 * P : (m0 + 1) * P, ci * 512 : (ci + 1) * 512])
                nc.gpsimd.collective_compute(
                    "AllGather", mybir.AluOpType.bypass,
                    replica_groups=pair_groups,
                    ins=[cc_in[:].opt()], outs=[cc_out[:].opt()],
                )

            def emit_vall_fill(l, hh, mlo, mhi):
                # fill v_all slots [hh*MH+mlo, hh*MH+mhi) of both ranks from
                # the gather output (scalar queue: never upstream of staging)
                if (l, "v_all") not in S_:
                    S_[l, "v_all"] = sb.tile([P, NS, D], BF16, tag="vall", name=f"vall{l}")
                cc_out = S_[l, "cc_out_v", hh]
                for r in range(2):
                    eng = nc.sync if r == 0 else nc.scalar
                    eng.dma_start(
                        S_[l, "v_all"][:, r * NJ + hh * MH + mlo : r * NJ + hh * MH + mhi, :],
                        cc_out[r, mlo * P : mhi * P, :].rearrange("(m p) d -> p m d", p=P),
                    )

            def emit_attention(l, hT, kts_src, fills_mid=(), fills_end=()):
                """scoresT -> max-free exp -> l -> attn@v -> h_attnT."""
                slot_order = [r * NJ + m for m in range(MH) for r in range(2)] + \
                             [r * NJ + m for m in range(MH, NJ) for r in range(2)]
                expT = sb.tile([P, NS, SO], BF16, tag="expT", name=f"expT{l}")
                l_ps = [ps_p.tile([1, 512], F32, tag="lps", bufs=2, name=f"lps{l}{c}")
                        for c in range(2)]

                if l == 1:
                    pe_warmup(70)
                unit_ctr = [0]
                for si, s in enumerate(slot_order):
                    if si == 8:
                        for f in fills_mid:
                            emit_vall_fill(*f)
                    r, m = s // NJ, s % NJ
                    kTs = sb.tile([P, ND, P], BF16, tag="kts", bufs=2, name=f"kts{l}{s}")
                    eng = nc.sync if si % 2 == 0 else nc.gpsimd
                    eng.dma_start(kTs[:], kts_src(r, m))
                    units = [(m // MH, (m % MH) * P)]
                    if m // MH == 0:
                        units.append((1, 0))
                    for (c, off) in units:
                        utag = "sc" if unit_ctr[0] % 2 == 0 else "mm"
                        unit_ctr[0] += 1
                        sc = ps_p.tile([P, 512], F32, tag=utag, bufs=2, name=f"sc{l}{s}{c}")
                        for k in range(ND):
                            nc.tensor.matmul(
                                sc[:, off:], kTs[:, k, :],
                                hT[:, k, c * 512 + off : (c + 1) * 512],
                                start=(k == 0), stop=(k == ND - 1),
                            )
                        if c == m // MH:
                            # suffix head block: causal mask (data per core)
                            nc.vector.tensor_add(
                                sc[:, off : off + P], sc[:, off : off + P],
                                mask_sb[:, r, :],
                            )
                        nc.scalar.activation(
                            expT[:, s, c * 512 + off : (c + 1) * 512], sc[:, off:],
                            mybir.ActivationFunctionType.Exp,
                        )
                        nc.tensor.matmul(
                            l_ps[c][:, off:], ones_sb[:],
                            expT[:, s, c * 512 + off : (c + 1) * 512],
                            start=(si == 0), stop=(si == len(slot_order) - 1),
                            skip_group_check=True,
                        )

                for f in fills_end:
                    emit_vall_fill(*f)

                inv_bc = []
                for c in range(2):
                    inv_sb = sb.tile([1, 512], F32, tag="inv", bufs=1, name=f"inv{l}{c}")
                    nc.vector.reciprocal(inv_sb[:], l_ps[c][:])
                    bc = sb.tile([P, 512], F32, tag="invbc", bufs=2, name=f"invbc{l}{c}")
                    nc.gpsimd.partition_broadcast(bc[:], inv_sb[:])
                    inv_bc.append(bc)

                v_all = S_[l, "v_all"]
                h_attnT = sb.tile([P, ND, SO], BF16, tag="hat", name=f"hat{l}")
                for c in range(2):
                    slots_c = [s for s in slot_order if (s % NJ) // MH <= c]
                    for i in range(ND):
                        av = ps_p.tile([P, 512], F32, tag="av", bufs=2, name=f"av{l}{c}{i}")
                        for sj, s in enumerate(slots_c):
                            m = s % NJ
                            off = (m % MH) * P if (m // MH) == c else 0
                            nc.tensor.matmul(
                                av[:, off:],
                                v_all[:, s, i * P : (i + 1) * P],
                                expT[:, s, c * 512 + off : (c + 1) * 512],
                                start=(sj == 0), stop=(sj == len(slots_c) - 1),
                                skip_group_check=True,
                            )
                        nc.vector.tensor_mul(
                            h_attnT[:, i, c * 512 : (c + 1) * 512], av[:], inv_bc[c][:],
                        )
                return h_attnT

            def emit_wo(l, hT, h_attnT, wo_ap):
                h_resT = sb.tile([P, ND, SO], BF16, tag="hres", name=f"hres{l}")
                for i in range(ND):
                    for c in range(2):
                        po = ps_p.tile([P, 512], F32, tag="mm", bufs=2, name=f"po{l}{i}{c}")
                        for k in range(ND):
                            nc.tensor.matmul(
                                po[:], wo_ap[:, k, i * P : (i + 1) * P],
                                h_attnT[:, k, c * 512 : (c + 1) * 512],
                                start=(k == 0), stop=(k == ND - 1),
                            )
                        nc.vector.tensor_add(
                            h_resT[:, i, c * 512 : (c + 1) * 512], po[:],
                            hT[:, i, c * 512 : (c + 1) * 512],
                        )
                return h_resT

            def emit_ffn_chunk(l, c, h_resT, own_next, last_col=None,
                               pre_w2=None, post_w1=None, post_w2_i=None,
                               mid_w1=None):
                qoff = c * 512
                midT = sb.tile([P, NH, 512], BF16, tag="vall", name=f"midT{l}{c}")
                for ch in range(NH // W1CH):
                    if ch == 2 and mid_w1 is not None:
                        mid_w1()
                    w1_sb = sb.tile([P, ND, W1CH * P], BF16, tag="wkv", bufs=2,
                                    name=f"w1_{l}{c}{ch}")
                    nc.sync.dma_start(
                        w1_sb[:],
                        wts[l, "w1"][:, ch * W1CH * P : (ch + 1) * W1CH * P]
                        .rearrange("(k p) n -> p k n", p=P),
                    )
                    for hb in range(W1CH):
                        g = ch * W1CH + hb
                        pm = ps_p.tile([P, 512], F32, tag="mm", bufs=2,
                                       name=f"pm{l}{c}{g}")
                        for k in range(ND):
                            nc.tensor.matmul(
                                pm[:], w1_sb[:, k, hb * P : (hb + 1) * P],
                                h_resT[:, k, qoff : qoff + 512],
                                start=(k == 0), stop=(k == ND - 1),
                            )
                        nc.vector.tensor_scalar_max(midT[:, g, :], pm[:], 0.0)
                if post_w1 is not None:
                    post_w1()
                if pre_w2 is not None:
                    pre_w2()
                for i in range(ND):
                    w2_sb = sb.tile([P, NH, P], BF16, tag="kts", bufs=2,
                                    name=f"w2_{l}{c}{i}")
                    nc.gpsimd.dma_start(
                        w2_sb[:],
                        wts[l, "w2"][:, i * P : (i + 1) * P]
                        .rearrange("(k p) n -> p k n", p=P),
                    )
                    pw = ps_p.tile([P, 512], F32, tag="mm", bufs=2, name=f"pw{l}{c}{i}")
                    for hb in range(NH):
                        nc.tensor.matmul(
                            pw[:], w2_sb[:, hb, :], midT[:, hb, :],
                            start=(hb == 0), stop=(hb == NH - 1),
                        )
                    nc.vector.tensor_add(
                        own_next[:, i, qoff : qoff + 512], pw[:],
                        h_resT[:, i, qoff : qoff + 512],
                    )
                    if last_col is not None:
                        nc.vector.tensor_add(
                            last_col[:, i : i + 1], pw[:, 511:512],
                            h_resT[:, i, SO - 1 : SO],
                        )
                    if post_w2_i is not None:
                        post_w2_i(i)

            # ================= block 1 =================
            emit_cc_tiles(1)
            emit_kT_half(1, 0, own_hT, wk_sb)
            emit_v_tiles(1, 0, 1, own_hT, wv_sb)
            emit_kT_half(1, 1, own_hT, wk_sb)
            emit_v_tiles(1, 1, 1, own_hT, wv_sb)
            if stage == "kv":
                for s in range(NS):
                    r, m = s // NJ, s % NJ
                    nc.sync.dma_start(
                        dbg[:, :, s * P : (s + 1) * P],
                        S_[1, "cc_out_k"][m // MH][r, :, (m % MH) * P : (m % MH + 1) * P]
                        .rearrange("(i p) t -> p i t", p=P),
                    )
            elif stage in ("att", "block1", "full"):
                k1half = S_[1, "cc_out_k"]
                h_attnT = emit_attention(
                    1, own_hT,
                    lambda r, m: k1half[m // MH][r, :, (m % MH) * P : (m % MH + 1) * P]
                    .rearrange("(i p) t -> p i t", p=P),
                    fills_mid=[(1, 0, 0, 4)],
                    fills_end=[(1, 1, 0, 4)])
                if stage == "att":
                    nc.sync.dma_start(dbg[:, :, :SO], h_attnT[:])
                else:
                    h_resT = emit_wo(1, own_hT, h_attnT, wo_sb)
                    # prefetch block-2 wo while the FFN runs
                    wo2_sb = sb.tile([P, ND, D], BF16, tag="wo", name="wo2")
                    nc.sync.dma_start(wo2_sb[:], wts[2, "wo"][:].rearrange("(k p) n -> p k n", p=P))
                    own2 = sb.tile([P, ND, SO], BF16, tag="own", bufs=2, name="own2")

                    wkv2 = sb.tile([P, 2, ND, D], BF16, tag="expT", name="wkv2")

                    def prefetch_wkv2():
                        # scalar queue: idle during the FFN (exps all done),
                        # so this 4MB load contends with nothing
                        for k in range(ND):
                            nc.scalar.dma_start(wkv2[:, 0, k, :], wts[2, "wk"][k * P : (k + 1) * P, :])
                            nc.scalar.dma_start(wkv2[:, 1, k, :], wts[2, "wv"][k * P : (k + 1) * P, :])

                    emit_ffn_chunk(1, 0, h_resT, own2, pre_w2=prefetch_wkv2)
                    if stage == "full":
                        # block-2 K/V first half mid-FFN: its AllGathers run
                        # under block-1's remaining FFN work
                        emit_cc_tiles(2)
                        emit_kT_half(2, 0, own2, wkv2[:, 0])
                        emit_v_tiles(2, 0, 1, own2, wkv2[:, 1])
                    emit_ffn_chunk(1, 1, h_resT, own2)
                    if stage == "block1":
                        nc.sync.dma_start(dbg[:, :, :SO], own2[:])

            # ================= block 2 =================
            if stage == "full":
                emit_vall_fill(2, 0, 0, 4)
                emit_kT_half(2, 1, own2, wkv2[:, 0])
                emit_v_tiles(2, 1, 1, own2, wkv2[:, 1])
                k2half = S_[2, "cc_out_k"]
                h_attnT2 = emit_attention(
                    2, own2,
                    lambda r, m: k2half[m // MH][r, :, (m % MH) * P : (m % MH + 1) * P]
                    .rearrange("(i p) t -> p i t", p=P),
                    fills_end=[(2, 1, 0, 4)])
                h_resT2 = emit_wo(2, own2, h_attnT2, wo2_sb)
                own3 = sb.tile([P, ND, SO], BF16, tag="own", bufs=2, name="own3")
                last_col = sb.tile([P, ND], BF16, tag="lastcol")

                # FFN half with the final token first; AllGather of the last
                # token's activations overlaps the other half
                emit_ffn_chunk(2, 1, h_resT2, own3, last_col=last_col)
                lc_t = sb.tile([ND, P], BF16, tag="lct")
                ps_lc = ps_p.tile([P, P], BF16, tag="sc", bufs=2, name="pslc")
                nc.tensor.transpose(ps_lc[:ND, :P], last_col[:], ident[:])
                nc.vector.tensor_copy(lc_t[:], ps_lc[:ND, :P])
                cc_l_in = dram_p.tile([D], BF16, tag="ccl")
                cc_l_out = dram_p.tile([n_cores, D], BF16, tag="cclo")
                nc.sync.dma_start(cc_l_in[:].rearrange("(i p) -> i p", p=P), lc_t[:])
                nc.gpsimd.collective_compute(
                    "AllGather", mybir.AluOpType.bypass,
                    replica_groups=all_group,
                    ins=[cc_l_in[:].opt()], outs=[cc_l_out[:].opt()],
                )
                # gather read issued ahead of the 4MB w_out stream on the
                # scalar queue so the lhsT transposes never stall
                h_last = sb.tile([4, ND, P], BF16, tag="hlast")
                nc.scalar.dma_start(
                    h_last[:],
                    cc_l_out[:].rearrange("r (i p) -> r i p", p=P)[1::2],
                )
                S_["h_last"] = h_last

                lhsT = sb.tile([P, ND, 4], BF16, tag="lhsT")
                lg_ctr = [0]

                def logits_chunks(lo, hi, wo_stream):
                    for vc in range(lo, hi):
                        pl = ps_p.tile([P, 512], F32, tag="av", bufs=2, name=f"pl{vc}")
                        for k in range(ND):
                            nc.tensor.matmul(
                                pl[:4, :VC], lhsT[:, k, :],
                                wo_stream[:, k, (vc - lo) * VC : (vc - lo + 1) * VC],
                                start=(k == 0), stop=(k == ND - 1),
                            )
                        lg = sb.tile([4, VC], F32, tag="lg", bufs=1, name=f"lg{vc}")
                        nc.vector.tensor_copy(lg[:], pl[:4, :VC])
                        nc.scalar.dma_start(logits[:, vc * VC : (vc + 1) * VC], lg[:])

                wo_ta = sb.tile([P, ND, 4 * VC], BF16, tag="expT", name="wo_ta")
                nc.scalar.dma_start(
                    wo_ta[:],
                    w_out[:, : 4 * VC].rearrange("(k p) n -> p k n", p=P),
                )
                wo_tb = None

                def logits_mid():
                    # lhsT prep + first half of the vocab chunks, interleaved
                    # between w1 and w2 of the last FFN chunk (the 8-core
                    # AllGather is long done by now)
                    nonlocal wo_tb
                    h_last = S_["h_last"]
                    for i in range(ND):
                        ps_t = ps_p.tile([P, 512], BF16, tag="sc", bufs=2, name=f"pst{i}")
                        nc.tensor.transpose(ps_t[:, :4], h_last[:, i, :], ident[:4, :4])
                        nc.vector.tensor_copy(lhsT[:, i, :], ps_t[:, :4])
                    logits_chunks(0, 4, wo_ta)
                    wo_tb = sb.tile([P, ND, 4 * VC], BF16, tag="expT", name="wo_tb")
                    nc.scalar.dma_start(
                        wo_tb[:],
                        w_out[:, 4 * VC :].rearrange("(k p) n -> p k n", p=P),
                    )

                def logits_late(i):
                    if i == 5:
                        logits_chunks(4, NVC, wo_tb)

                emit_ffn_chunk(2, 0, h_resT2, own3, mid_w1=logits_mid,
                               post_w2_i=logits_late)

    nc.compile()
    return nc


# ----------------------------------------------------------------------------
# host side
# ----------------------------------------------------------------------------

def make_in_maps(tokens, emb, pe, weights, S=2048, D=1024, H=4096, V=32000,
                 n_cores=8):
    """weights: dict with l{1,2}_{wk,wv,wo,w1,w2} and w_out (fp32 numpy)."""
    bf = ml_dtypes.bfloat16
    NJ = (S // P) // 2
    ND = D // P
    SO = S // 2
    VS = V // n_cores
    emb_f = np.ascontiguousarray(emb, dtype=np.float32)
    pe_f = np.asarray(pe, dtype=np.float32)
    scale = 1.0 / np.sqrt(float(D))
    w_bf = {}
    for l in (1, 2):
        w_bf[f"l{l}_wk"] = (np.asarray(weights[f"l{l}_wk"], np.float32) * scale).astype(bf)
        for nm in ("wv", "wo", "w1", "w2"):
            w_bf[f"l{l}_{nm}"] = np.asarray(weights[f"l{l}_{nm}"], np.float32).astype(bf)
    w_out_bf = np.asarray(weights["w_out"], np.float32).astype(bf)

    tokens = np.asarray(tokens)
    in_maps = []
    # scoresT head-block masks: [t_row, q_col] within the diagonal 128-tile;
    # allowed iff q >= t  ->  NEG on the strict lower triangle
    tri = np.tril(np.full((P, P), NEG, np.float32), k=-1)
    for c in range(n_cores):
        b, t = c // 2, c % 2
        own_rows = np.concatenate(
            [np.arange((2 * j + t) * P, (2 * j + t + 1) * P) for j in range(NJ)]
        )
        h0 = (emb_f[tokens[b, own_rows]] + pe_f[own_rows]).astype(np.float32)
        h0T = np.ascontiguousarray(h0.T).reshape(ND, P, SO).astype(bf)
        # head-block mask per stored rank r: global q-tile 2m+t vs key tile
        # 2m+r: t==r -> diagonal triangle; t<r -> fully masked; t>r -> allowed
        maskT = np.zeros((2, P, P), np.float32)
        for r in range(2):
            if t == r:
                maskT[r] = tri
            elif t < r:
                maskT[r] = NEG
        in_map = {
            "h0T": h0T,
            "maskT": maskT.astype(bf),
            "w_out": np.ascontiguousarray(w_out_bf[:, c * VS : (c + 1) * VS]),
        }
        in_map.update(w_bf)
        in_maps.append(in_map)
    return in_maps


_NC_CACHE = {}


def _get_nc(key=(2048, 1024, 4096, 32000, 8)):
    if key not in _NC_CACHE:
        _NC_CACHE[key] = build_nc(*key)
    return _NC_CACHE[key]


def kernel(tokens, emb, pe, l1_wk, l1_wv, l1_wo, l1_w1, l1_w2,
           l2_wk, l2_wv, l2_wo, l2_w1, l2_w2, w_out):
    S = int(np.asarray(tokens).shape[1])
    D = int(np.asarray(emb).shape[1])
    H = int(np.asarray(l1_w1).shape[1])
    V = int(np.asarray(emb).shape[0])
    n_cores = 8
    nc = _get_nc((S, D, H, V, n_cores))
    weights = dict(
        l1_wk=l1_wk, l1_wv=l1_wv, l1_wo=l1_wo, l1_w1=l1_w1, l1_w2=l1_w2,
        l2_wk=l2_wk, l2_wv=l2_wv, l2_wo=l2_wo, l2_w1=l2_w1, l2_w2=l2_w2,
        w_out=w_out,
    )
    in_maps = make_in_maps(tokens, emb, pe, weights, S, D, H, V, n_cores)
    try:
        res = run_bass_kernel_spmd(nc, in_maps, core_ids=list(range(n_cores)))
    except Exception:
        # a previous crashed run can leave the device wedged; one retry
        # (fresh NRT session) clears it
        import os
        os.environ.setdefault("NEURON_RT_RESET_CORES", "1")
        res = run_bass_kernel_spmd(nc, in_maps, core_ids=list(range(n_cores)))
    VS = V // n_cores
    out = np.zeros((np.asarray(tokens).shape[0], V), np.float32)
    for c in range(n_cores):
        out[:, c * VS : (c + 1) * VS] = res.results[c]["logits"]
    return out
